# revision 2
# baseline (speedup 1.0000x reference)
"""Cross-conditional GPT2 sparse attention block on 8 Trainium2 NeuronCores.

Sharding: core = (batch b in 0..3) x (head-group g in 0..1, 6 heads each).
Each core computes, for its (b, g):
  qT/kT = (Wq_g @ x_b^T + bq_g)  laid out [d_on_partitions, L]
  v     = x_b @ Wv_g^T + bv_g    natural layout [L, 384], interleaved with a
          ones column per head ([L, 6, 65]) so att@v also yields the softmax
          denominator for free.
  scores are computed *transposed* (sT[j, i]) so that softmax needs no
  transpose at all: exp on ACT, multiplicative 0/1 mask (host-built, bf16),
  att@v via lhsT=v (natural layout), denominator broadcast across partitions
  via a K=1 PE matmul, then the partial output projection with Wp[:, g]^T.
Host sums the two per-batch partials and adds bp.
"""

import sys

sys.path.insert(0, "/opt/trn_rl_repo")

from contextlib import ExitStack

import ml_dtypes
import numpy as np

import concourse.bacc as bacc
import concourse.bass as bass
import concourse.mybir as mybir
import concourse.tile as tile
from concourse.bass_utils import run_bass_kernel_spmd

# ---- problem constants (hardcoded per spec) ----
B = 4
T = 512
N = 8
C = 768
NHEAD = 12
L = 3 * T + 4 * N  # 1568
P = 128
G = C // 2  # 384 channels per head-group
NH = 6  # heads per core
D = 64  # head dim
ET = C // P  # 6 e-tiles (contraction of x @ W)
CT = G // P  # 3 c-tiles of the group's channels
NJT = (L + P - 1) // P  # 13 j tiles (12x128 + 32)
JPAD = NJT * P  # 1664
I_CHUNKS = [(0, 512), (512, 512), (1024, 512), (1536, 32)]
SCALE = 1.0 / 8.0  # 1/sqrt(64)

F32 = mybir.dt.float32
BF16 = mybir.dt.bfloat16

_NC = None  # cached compiled Bass program


def _jl(jt):
    return P if jt < NJT - 1 else L - (NJT - 1) * P  # 128 or 32


def _build_program():
    nc = bacc.Bacc("TRN2", target_bir_lowering=False, debug=False)

    xT_d = nc.dram_tensor("xT", [C, L], F32, kind="ExternalInput")
    wq_d = nc.dram_tensor("wqT", [C, G], F32, kind="ExternalInput")
    wk_d = nc.dram_tensor("wkT", [C, G], F32, kind="ExternalInput")
    wv_d = nc.dram_tensor("wvT", [C, G], F32, kind="ExternalInput")
    wp_d = nc.dram_tensor("wpT", [G, C], F32, kind="ExternalInput")
    bq_d = nc.dram_tensor("bqP", [P, CT], F32, kind="ExternalInput")
    bk_d = nc.dram_tensor("bkP", [P, CT], F32, kind="ExternalInput")
    bv_d = nc.dram_tensor("bvB", [P, G], F32, kind="ExternalInput")
    mask_d = nc.dram_tensor("maskT", [JPAD, L], BF16, kind="ExternalInput")
    out_d = nc.dram_tensor("out_part", [L, C], F32, kind="ExternalOutput")

    with tile.TileContext(nc) as tc, ExitStack() as big:
        persist = big.enter_context(tc.tile_pool(name="persist", bufs=1))

        # persistent SBUF tensors
        qT = persist.tile([P, CT, L], F32, name="qT")
        kT = persist.tile([P, CT, L], F32, name="kT")
        v_ones = persist.tile([P, NJT, NH, D + 1], BF16, name="v_ones")
        maskT = persist.tile([P, NJT, L], BF16, name="maskT_sb")
        yT = persist.tile([P, CT, L], F32, name="yT")
        wp_sb = persist.tile([P, CT, C], F32, name="wp_sb")
        ones64 = persist.tile([1, D], F32, name="ones64")
        bv_sb = persist.tile([P, G], F32, name="bv_sb")

        nc.sync.dma_start(maskT[:], mask_d.rearrange("(jt p) i -> p jt i", p=P))
        nc.sync.dma_start(wp_sb[:], wp_d.rearrange("(ct p) n -> p ct n", p=P))
        nc.sync.dma_start(bv_sb[:], bv_d[:])
        nc.gpsimd.memset(ones64[:], 1.0)
        nc.gpsimd.memset(v_ones[:], 1.0)

        # ---------- Phase A: projections ----------
        with (
            tc.tile_pool(name="phA", bufs=1) as phA,
            tc.tile_pool(name="psA", bufs=2, space="PSUM") as psA,
        ):
            xT = phA.tile([P, ET, L], F32, name="xT_sb")
            wq_sb = phA.tile([P, ET, G], F32, name="wq_sb")
            wk_sb = phA.tile([P, ET, G], F32, name="wk_sb")
            wv_sb = phA.tile([P, ET, G], F32, name="wv_sb")
            bq_sb = phA.tile([P, CT], F32, name="bq_sb")
            bk_sb = phA.tile([P, CT], F32, name="bk_sb")

            nc.sync.dma_start(xT[:], xT_d.rearrange("(et p) i -> p et i", p=P))
            nc.sync.dma_start(wq_sb[:], wq_d.rearrange("(et p) m -> p et m", p=P))
            nc.sync.dma_start(wk_sb[:], wk_d.rearrange("(et p) m -> p et m", p=P))
            nc.sync.dma_start(wv_sb[:], wv_d.rearrange("(et p) m -> p et m", p=P))
            nc.sync.dma_start(bq_sb[:], bq_d[:])
            nc.sync.dma_start(bk_sb[:], bk_d[:])

            # qT / kT: out[c_tile, i] accumulated over e tiles
            for dst, w_sb, b_sb in ((qT, wq_sb, bq_sb), (kT, wk_sb, bk_sb)):
                for ct in range(CT):
                    for i0, ilen in I_CHUNKS:
                        ps = psA.tile([P, 512], F32, name="ps_qk", tag="ps_qk")
                        for et in range(ET):
                            nc.tensor.matmul(
                                ps[:, :ilen],
                                w_sb[:, et, ct * P : (ct + 1) * P],
                                xT[:, et, i0 : i0 + ilen],
                                start=(et == 0),
                                stop=(et == ET - 1),
                            )
                        nc.vector.tensor_scalar(
                            dst[:, ct, i0 : i0 + ilen],
                            ps[:, :ilen],
                            b_sb[:, ct : ct + 1],
                            None,
                            mybir.AluOpType.add,
                        )

            # v natural layout [i, 384] + bias, into the 65-strided bf16 buffer
            for it in range(NJT):
                il = _jl(it)
                ps = psA.tile([P, G], F32, name="ps_v", tag="ps_v")
                for et in range(ET):
                    nc.tensor.matmul(
                        ps[:il, :],
                        xT[:, et, it * P : it * P + il],
                        wv_sb[:, et, :],
                        start=(et == 0),
                        stop=(et == ET - 1),
                    )
                nc.vector.tensor_tensor(
                    v_ones[:il, it, :, 0:D],
                    ps[:il, :].rearrange("p (h d) -> p h d", h=NH),
                    bv_sb[:il, :].rearrange("p (h d) -> p h d", h=NH),
                    mybir.AluOpType.add,
                )

        # ---------- Phase B: attention per head ----------
        with (
            tc.tile_pool(name="phB", bufs=1) as phB,
            tc.tile_pool(name="psS", bufs=3, space="PSUM") as psS,
            tc.tile_pool(name="psY", bufs=2, space="PSUM") as psY,
            tc.tile_pool(name="psBC", bufs=2, space="PSUM") as psBC,
        ):
            for h in range(NH):
                pof = D * (h % 2)
                ct = h // 2
                for i0, ilen in I_CHUNKS:
                    pts = []
                    for jt in range(NJT):
                        jl = _jl(jt)
                        ps_s = psS.tile([P, 512], F32, name="ps_s", tag="ps_s")
                        nc.tensor.matmul(
                            ps_s[:jl, :ilen],
                            kT[pof : pof + D, ct, jt * P : jt * P + jl],
                            qT[pof : pof + D, ct, i0 : i0 + ilen],
                            start=True,
                            stop=True,
                        )
                        pt = phB.tile([P, 512], BF16, name="pT", tag="pT", bufs=26)
                        nc.scalar.activation(
                            pt[:jl, :ilen],
                            ps_s[:jl, :ilen],
                            mybir.ActivationFunctionType.Exp,
                            bias=0.0,
                            scale=SCALE,
                        )
                        nc.vector.tensor_tensor(
                            pt[:jl, :ilen],
                            pt[:jl, :ilen],
                            maskT[:jl, jt, i0 : i0 + ilen],
                            mybir.AluOpType.mult,
                        )
                        pts.append(pt)

                    ps_y = psY.tile([P, 512], F32, name="ps_y", tag="ps_y")
                    for jt in range(NJT):
                        jl = _jl(jt)
                        nc.tensor.matmul(
                            ps_y[: D + 1, :ilen],
                            v_ones[:jl, jt, h, :],
                            pts[jt][:jl, :ilen],
                            start=(jt == 0),
                            stop=(jt == NJT - 1),
                        )

                    recip = phB.tile([1, 512], F32, name="recip", tag="recip", bufs=3)
                    nc.vector.reciprocal(recip[0:1, :ilen], ps_y[D : D + 1, :ilen])
                    ps_bc = psBC.tile([D, 512], F32, name="ps_bc", tag="ps_bc")
                    nc.tensor.matmul(
                        ps_bc[:, :ilen],
                        ones64[0:1, :],
                        recip[0:1, :ilen],
                        start=True,
                        stop=True,
                    )
                    bc_sb = phB.tile([D, 512], F32, name="bc_sb", tag="bc_sb", bufs=3)
                    nc.any.tensor_copy(bc_sb[:, :ilen], ps_bc[:, :ilen])
                    nc.vector.tensor_tensor(
                        yT[pof : pof + D, ct, i0 : i0 + ilen],
                        ps_y[0:D, :ilen],
                        bc_sb[:, :ilen],
                        mybir.AluOpType.mult,
                    )

        # ---------- Phase C: output projection (partial) ----------
        with (
            tc.tile_pool(name="phC", bufs=3) as phC,
            tc.tile_pool(name="psC", bufs=2, space="PSUM") as psC,
        ):
            for it in range(NJT):
                il = _jl(it)
                o_sb = phC.tile([P, C], F32, name="o_sb", tag="o_sb")
                for nch in range(2):
                    ps_o = psC.tile([P, 384], F32, name="ps_o", tag="ps_o")
                    for kt in range(CT):
                        nc.tensor.matmul(
                            ps_o[:il, :],
                            yT[:, kt, it * P : it * P + il],
                            wp_sb[:, kt, nch * 384 : (nch + 1) * 384],
                            start=(kt == 0),
                            stop=(kt == CT - 1),
                        )
                    nc.any.tensor_copy(o_sb[:il, nch * 384 : (nch + 1) * 384], ps_o[:il, :])
                nc.sync.dma_start(out_d[it * P : it * P + il, :], o_sb[:il, :])

    nc.compile()
    return nc


def _build_mask_np(seg_starts, seg_ends):
    """True = masked. Mirrors reference._build_mask in numpy."""
    ML = 3 * T
    tril = np.tril(np.ones((T, T), dtype=bool))
    sl = np.tril(np.ones((T, T), dtype=bool), -1)
    m = np.zeros((L, L), dtype=bool)
    m[:ML, :ML] = True
    m[0:T, 0:T] = ~tril
    m[T : 2 * T, 0:T] = ~tril
    m[T : 2 * T, T : 2 * T] = ~sl
    m[T : 2 * T, 2 * T : 3 * T] = ~sl
    m[2 * T : 3 * T, 0:T] = ~tril
    m[2 * T : 3 * T, T : 2 * T] = ~tril
    m[2 * T : 3 * T, 2 * T : 3 * T] = ~sl
    m[:ML, ML:] = True
    frames = np.arange(T)[None, :, None]
    allowed = (frames >= seg_starts[:, None, :]) & (frames < seg_ends[:, None, :])
    mask = np.broadcast_to(m[None], (B, L, L)).copy()
    for row0, col_blocks in ((T, (0, 2, 3)), (2 * T, (1, 2, 3))):
        for j in col_blocks:
            c0 = ML + j * N
            mask[:, row0 : row0 + T, c0 : c0 + N] &= ~allowed
    return mask


def get_nc():
    global _NC
    if _NC is None:
        _NC = _build_program()
    return _NC


def make_in_maps(x, Wq, bq, Wk, bk, Wv, bv, Wp, bp, seg_starts, seg_ends):
    mask = _build_mask_np(np.asarray(seg_starts), np.asarray(seg_ends))
    in_maps = []
    for core in range(8):
        b, g = core // 2, core % 2
        gs = slice(g * G, (g + 1) * G)
        mT = np.zeros((JPAD, L), dtype=ml_dtypes.bfloat16)
        mT[:L, :] = (~mask[b]).T.astype(ml_dtypes.bfloat16)
        in_maps.append(
            {
                "xT": np.ascontiguousarray(x[b].T),
                "wqT": np.ascontiguousarray(Wq[gs, :].T),
                "wkT": np.ascontiguousarray(Wk[gs, :].T),
                "wvT": np.ascontiguousarray(Wv[gs, :].T),
                "wpT": np.ascontiguousarray(Wp[:, gs].T),
                "bqP": np.ascontiguousarray(bq[gs].reshape(CT, P).T),
                "bkP": np.ascontiguousarray(bk[gs].reshape(CT, P).T),
                "bvB": np.broadcast_to(bv[gs], (P, G)).copy(),
                "maskT": mT,
            }
        )
    return in_maps


def kernel(x, Wq, bq, Wk, bk, Wv, bv, Wp, bp, seg_starts, seg_ends, T_motion=None,
           N=None, _trace=False, **_unused):
    x = np.asarray(x, np.float32)
    args = [np.asarray(a, np.float32) for a in (Wq, bq, Wk, bk, Wv, bv, Wp, bp)]
    Wq, bq, Wk, bk, Wv, bv, Wp, bp = args
    nc = get_nc()
    in_maps = make_in_maps(x, Wq, bq, Wk, bk, Wv, bv, Wp, bp, seg_starts, seg_ends)
    res = run_bass_kernel_spmd(nc, in_maps, core_ids=list(range(8)), trace=_trace)
    parts = [r["out_part"] for r in res.results]
    y = np.empty((B, L, C), np.float32)
    for b in range(B):
        y[b] = parts[2 * b] + parts[2 * b + 1] + bp
    if _trace:
        kernel.last_results = res
    return y


# revision 8
# speedup vs baseline: 1.2359x; 1.2359x over previous
"""Cross-conditional GPT2 sparse attention block on 8 Trainium2 NeuronCores.

Sharding: core = (batch b in 0..3) x (head-group g in 0..1, 6 heads each).
Each core computes, for its (b, g):
  qT/kT = (Wq_g @ x_b^T + bq_g)  laid out [d_on_partitions, L]
  v     = x_b @ Wv_g^T + bv_g    natural layout [L, 384], interleaved with a
          ones column per head ([L, 6, 65]) so att@v also yields the softmax
          denominator for free.
  scores are computed *transposed* (sT[j, i]) so that softmax needs no
  transpose at all: exp on ACT, multiplicative 0/1 mask (host-built, bf16),
  att@v via lhsT=v (natural layout), denominator broadcast across partitions
  via a K=1 PE matmul, then the partial output projection with Wp[:, g]^T.
Host sums the two per-batch partials and adds bp.
"""

import sys

sys.path.insert(0, "/opt/trn_rl_repo")

from contextlib import ExitStack

import ml_dtypes
import numpy as np

import concourse.bacc as bacc
import concourse.bass as bass
import concourse.mybir as mybir
import concourse.tile as tile
from concourse.bass_utils import run_bass_kernel_spmd

# ---- problem constants (hardcoded per spec) ----
B = 4
T = 512
N = 8
C = 768
NHEAD = 12
L = 3 * T + 4 * N  # 1568
P = 128
G = C // 2  # 384 channels per head-group
NH = 6  # heads per core
D = 64  # head dim
ET = C // P  # 6 e-tiles (contraction of x @ W)
CT = G // P  # 3 c-tiles of the group's channels
NJT = (L + P - 1) // P  # 13 j tiles (12x128 + 32)
JPAD = NJT * P  # 1664
I_CHUNKS = [(0, 512), (512, 512), (1024, 512), (1536, 32)]
SCALE = 1.0 / 8.0  # 1/sqrt(64)

F32 = mybir.dt.float32
BF16 = mybir.dt.bfloat16

_NC = None  # cached compiled Bass program


def _jl(jt):
    return P if jt < NJT - 1 else L - (NJT - 1) * P  # 128 or 32


def _score_intervals(jt):
    """i-ranges (start, len) that can attend any column in j-tile jt.
    Derived from the cross-conditional mask block structure."""
    if jt <= 3:
        j0 = jt * P
        return [(j0, 512 - j0), (512 + j0, 512 - j0), (1024 + j0, 512 - j0), (1536, 32)]
    if jt <= 11:
        f0 = (jt % 4) * P
        return [(512 + f0, 512 - f0), (1024 + f0, 512 - f0), (1536, 32)]
    return [(512, 512), (1024, 512), (1536, 32)]


def _ich_of(a):
    return 3 if a == 1536 else a // 512


_ATTV_LAST = {0: 3, 1: NJT - 1, 2: NJT - 1, 3: NJT - 1}  # last jt per ich


def _build_program():
    nc = bacc.Bacc("TRN2", target_bir_lowering=False, debug=False)

    xT_d = nc.dram_tensor("xT", [C, L], F32, kind="ExternalInput")
    wq_d = nc.dram_tensor("wqT", [C, G], F32, kind="ExternalInput")
    wk_d = nc.dram_tensor("wkT", [C, G], F32, kind="ExternalInput")
    wv_d = nc.dram_tensor("wvT", [C, G], F32, kind="ExternalInput")
    wp_d = nc.dram_tensor("wpT", [G, C], F32, kind="ExternalInput")
    bq_d = nc.dram_tensor("bqP", [P, CT], F32, kind="ExternalInput")
    bk_d = nc.dram_tensor("bkP", [P, CT], F32, kind="ExternalInput")
    bv_d = nc.dram_tensor("bvB", [P, G], F32, kind="ExternalInput")
    maskd_d = nc.dram_tensor("maskD", [P, 2, P], BF16, kind="ExternalInput")
    maskt_d = nc.dram_tensor("maskTxt", [32, 1024], BF16, kind="ExternalInput")
    out_d = nc.dram_tensor("out_part", [L, C], F32, kind="ExternalOutput")

    with tile.TileContext(nc) as tc, ExitStack() as big:
        persist = big.enter_context(tc.tile_pool(name="persist", bufs=1))

        # persistent SBUF tensors
        qT = persist.tile([P, CT, L], F32, name="qT")
        kT = persist.tile([P, CT, L], F32, name="kT")
        v_ones = persist.tile([P, NJT, NH, D + 1], BF16, name="v_ones")
        maskD = persist.tile([P, 2, P], BF16, name="maskD_sb")
        maskTx = persist.tile([32, 1024], BF16, name="maskTx_sb")
        yT = persist.tile([P, CT, L], F32, name="yT")
        wp_sb = persist.tile([P, CT, C], F32, name="wp_sb")
        ones64 = persist.tile([1, D], F32, name="ones64")
        bv_sb = persist.tile([P, G], F32, name="bv_sb")

        nc.sync.dma_start(maskD[:], maskd_d[:])
        nc.sync.dma_start(maskTx[:], maskt_d[:])
        nc.sync.dma_start(wp_sb[:], wp_d.rearrange("(ct p) n -> p ct n", p=P))
        nc.sync.dma_start(bv_sb[:], bv_d[:])
        nc.gpsimd.memset(ones64[:], 1.0)
        nc.gpsimd.memset(v_ones[:], 1.0)

        # ---------- Phase A: projections ----------
        with (
            tc.tile_pool(name="phA", bufs=1) as phA,
            tc.tile_pool(name="psA", bufs=2, space="PSUM") as psA,
        ):
            xT = phA.tile([P, ET, L], F32, name="xT_sb")
            wq_sb = phA.tile([P, ET, G], F32, name="wq_sb")
            wk_sb = phA.tile([P, ET, G], F32, name="wk_sb")
            wv_sb = phA.tile([P, ET, G], F32, name="wv_sb")
            bq_sb = phA.tile([P, CT], F32, name="bq_sb")
            bk_sb = phA.tile([P, CT], F32, name="bk_sb")

            nc.sync.dma_start(xT[:], xT_d.rearrange("(et p) i -> p et i", p=P))
            nc.sync.dma_start(wq_sb[:], wq_d.rearrange("(et p) m -> p et m", p=P))
            nc.sync.dma_start(wk_sb[:], wk_d.rearrange("(et p) m -> p et m", p=P))
            nc.sync.dma_start(wv_sb[:], wv_d.rearrange("(et p) m -> p et m", p=P))
            nc.sync.dma_start(bq_sb[:], bq_d[:])
            nc.sync.dma_start(bk_sb[:], bk_d[:])

            # qT / kT: out[c_tile, i] accumulated over e tiles
            for dst, w_sb, b_sb in ((qT, wq_sb, bq_sb), (kT, wk_sb, bk_sb)):
                for ct in range(CT):
                    for i0, ilen in I_CHUNKS:
                        ps = psA.tile([P, 512], F32, name="ps_qk", tag="ps_qk")
                        for et in range(ET):
                            nc.tensor.matmul(
                                ps[:, :ilen],
                                w_sb[:, et, ct * P : (ct + 1) * P],
                                xT[:, et, i0 : i0 + ilen],
                                start=(et == 0),
                                stop=(et == ET - 1),
                            )
                        nc.vector.tensor_scalar(
                            dst[:, ct, i0 : i0 + ilen],
                            ps[:, :ilen],
                            b_sb[:, ct : ct + 1],
                            None,
                            mybir.AluOpType.add,
                        )

            # v natural layout [i, 384] + bias, into the 65-strided bf16 buffer
            for it in range(NJT):
                il = _jl(it)
                ps = psA.tile([P, G], F32, name="ps_v", tag="ps_v")
                for et in range(ET):
                    nc.tensor.matmul(
                        ps[:il, :],
                        xT[:, et, it * P : it * P + il],
                        wv_sb[:, et, :],
                        start=(et == 0),
                        stop=(et == ET - 1),
                    )
                nc.vector.tensor_tensor(
                    v_ones[:il, it, :, 0:D],
                    ps[:il, :].rearrange("p (h d) -> p h d", h=NH),
                    bv_sb[:il, :].rearrange("p (h d) -> p h d", h=NH),
                    mybir.AluOpType.add,
                )

        # ---------- Phase B: attention per head (jt-major, block-skipped) ----------
        with (
            tc.tile_pool(name="phB", bufs=1) as phB,
            tc.tile_pool(name="psS", bufs=3, space="PSUM") as psS,
            tc.tile_pool(name="psY", bufs=5, space="PSUM") as psY,
        ):
            for h in range(NH):
                pof = D * (h % 2)
                ct = h // 2
                ps_y = [
                    psY.tile([D + 1, 512], F32, name=f"ps_y{ich}", tag="ps_y")
                    for ich in range(4)
                ]
                started = [False] * 4
                for jt in range(NJT):
                    jl = _jl(jt)
                    for k, (a, ln) in enumerate(_score_intervals(jt)):
                        ps_s = psS.tile([P, 512], F32, name="ps_s", tag="ps_s")
                        nc.tensor.matmul(
                            ps_s[:jl, :ln],
                            kT[pof : pof + D, ct, jt * P : jt * P + jl],
                            qT[pof : pof + D, ct, a : a + ln],
                            start=True,
                            stop=True,
                        )
                        pt = phB.tile([P, 512], BF16, name="pT", tag="pT", bufs=14)
                        nc.scalar.activation(
                            pt[:jl, :ln],
                            ps_s[:jl, :ln],
                            mybir.ActivationFunctionType.Exp,
                            bias=0.0,
                            scale=SCALE,
                        )
                        # selective masking: diagonal 128-block or text-column strip
                        if jt <= 11 and k < (3 if jt <= 3 else 2):
                            # tril (T1) for U-cols always, and torso-rows @ L-cols;
                            # strictly-lower (T2) elsewhere (see reference mask)
                            didx = 0 if (jt <= 3 or (jt <= 7 and k == 1)) else 1
                            nc.vector.tensor_tensor(
                                pt[:jl, 0:P],
                                pt[:jl, 0:P],
                                maskD[:jl, didx, :],
                                mybir.AluOpType.mult,
                            )
                        elif jt == 12 and k < 2:
                            nc.vector.tensor_tensor(
                                pt[:jl, :ln],
                                pt[:jl, :ln],
                                maskTx[:jl, a - 512 : a - 512 + ln],
                                mybir.AluOpType.mult,
                            )
                        ich = _ich_of(a)
                        off = a - (0, 512, 1024, 1536)[ich]
                        nc.tensor.matmul(
                            ps_y[ich][:, off : off + ln],
                            v_ones[:jl, jt, h, :],
                            pt[:jl, :ln],
                            start=not started[ich],
                            stop=(jt == _ATTV_LAST[ich]),
                            skip_group_check=True,
                        )
                        started[ich] = True

                for ich, (i0, ilen) in enumerate(I_CHUNKS):
                    den = phB.tile([1, 512], F32, name="den", tag="den", bufs=4)
                    nc.scalar.activation(
                        den[0:1, :ilen],
                        ps_y[ich][D : D + 1, :ilen],
                        mybir.ActivationFunctionType.Copy,
                    )
                    ps_bc = psS.tile([D, 512], F32, name="ps_bc", tag="ps_s")
                    nc.tensor.matmul(
                        ps_bc[:, :ilen],
                        ones64[0:1, :],
                        den[0:1, :ilen],
                        start=True,
                        stop=True,
                    )
                    rc = phB.tile([D, 512], F32, name="rc", tag="rc", bufs=4)
                    nc.vector.reciprocal(rc[:, :ilen], ps_bc[:, :ilen])
                    nc.vector.tensor_tensor(
                        yT[pof : pof + D, ct, i0 : i0 + ilen],
                        ps_y[ich][0:D, :ilen],
                        rc[:, :ilen],
                        mybir.AluOpType.mult,
                    )

        # ---------- Phase C: output projection (partial) ----------
        with (
            tc.tile_pool(name="phC", bufs=3) as phC,
            tc.tile_pool(name="psC", bufs=2, space="PSUM") as psC,
        ):
            for it in range(NJT):
                il = _jl(it)
                o_sb = phC.tile([P, C], F32, name="o_sb", tag="o_sb")
                for nch in range(2):
                    ps_o = psC.tile([P, 384], F32, name="ps_o", tag="ps_o")
                    for kt in range(CT):
                        nc.tensor.matmul(
                            ps_o[:il, :],
                            yT[:, kt, it * P : it * P + il],
                            wp_sb[:, kt, nch * 384 : (nch + 1) * 384],
                            start=(kt == 0),
                            stop=(kt == CT - 1),
                        )
                    nc.any.tensor_copy(o_sb[:il, nch * 384 : (nch + 1) * 384], ps_o[:il, :])
                nc.sync.dma_start(out_d[it * P : it * P + il, :], o_sb[:il, :])

    nc.compile()
    return nc


def _build_mask_np(seg_starts, seg_ends):
    """True = masked. Mirrors reference._build_mask in numpy."""
    ML = 3 * T
    tril = np.tril(np.ones((T, T), dtype=bool))
    sl = np.tril(np.ones((T, T), dtype=bool), -1)
    m = np.zeros((L, L), dtype=bool)
    m[:ML, :ML] = True
    m[0:T, 0:T] = ~tril
    m[T : 2 * T, 0:T] = ~tril
    m[T : 2 * T, T : 2 * T] = ~sl
    m[T : 2 * T, 2 * T : 3 * T] = ~sl
    m[2 * T : 3 * T, 0:T] = ~tril
    m[2 * T : 3 * T, T : 2 * T] = ~tril
    m[2 * T : 3 * T, 2 * T : 3 * T] = ~sl
    m[:ML, ML:] = True
    frames = np.arange(T)[None, :, None]
    allowed = (frames >= seg_starts[:, None, :]) & (frames < seg_ends[:, None, :])
    mask = np.broadcast_to(m[None], (B, L, L)).copy()
    for row0, col_blocks in ((T, (0, 2, 3)), (2 * T, (1, 2, 3))):
        for j in col_blocks:
            c0 = ML + j * N
            mask[:, row0 : row0 + T, c0 : c0 + N] &= ~allowed
    return mask


def get_nc():
    global _NC
    if _NC is None:
        _NC = _build_program()
    return _NC


def make_in_maps(x, Wq, bq, Wk, bk, Wv, bv, Wp, bp, seg_starts, seg_ends):
    mask = _build_mask_np(np.asarray(seg_starts), np.asarray(seg_ends))
    r = np.arange(P)
    maskD = np.empty((P, 2, P), dtype=ml_dtypes.bfloat16)
    maskD[:, 0, :] = (r[:, None] <= r[None, :]).astype(ml_dtypes.bfloat16)  # tril.T
    maskD[:, 1, :] = (r[:, None] < r[None, :]).astype(ml_dtypes.bfloat16)  # strict
    in_maps = []
    for core in range(8):
        b, g = core // 2, core % 2
        gs = slice(g * G, (g + 1) * G)
        allowT = ~mask[b].T  # [j, i]
        maskTx = np.ascontiguousarray(
            allowT[1536:1568, 512:1536].astype(ml_dtypes.bfloat16)
        )
        in_maps.append(
            {
                "xT": np.ascontiguousarray(x[b].T),
                "wqT": np.ascontiguousarray(Wq[gs, :].T),
                "wkT": np.ascontiguousarray(Wk[gs, :].T),
                "wvT": np.ascontiguousarray(Wv[gs, :].T),
                "wpT": np.ascontiguousarray(Wp[:, gs].T),
                "bqP": np.ascontiguousarray(bq[gs].reshape(CT, P).T),
                "bkP": np.ascontiguousarray(bk[gs].reshape(CT, P).T),
                "bvB": np.broadcast_to(bv[gs], (P, G)).copy(),
                "maskD": maskD,
                "maskTxt": maskTx,
            }
        )
    return in_maps


def kernel(x, Wq, bq, Wk, bk, Wv, bv, Wp, bp, seg_starts, seg_ends, T_motion=None,
           N=None, _trace=False, **_unused):
    x = np.asarray(x, np.float32)
    args = [np.asarray(a, np.float32) for a in (Wq, bq, Wk, bk, Wv, bv, Wp, bp)]
    Wq, bq, Wk, bk, Wv, bv, Wp, bp = args
    nc = get_nc()
    in_maps = make_in_maps(x, Wq, bq, Wk, bk, Wv, bv, Wp, bp, seg_starts, seg_ends)
    res = run_bass_kernel_spmd(nc, in_maps, core_ids=list(range(8)), trace=_trace)
    parts = [r["out_part"] for r in res.results]
    y = np.empty((B, L, C), np.float32)
    for b in range(B):
        y[b] = parts[2 * b] + parts[2 * b + 1] + bp
    if _trace:
        kernel.last_results = res
    return y


# revision 9
# speedup vs baseline: 2.7622x; 2.2350x over previous
"""Cross-conditional GPT2 sparse attention block on 8 Trainium2 NeuronCores.

Sharding: core = (batch b in 0..3) x (head-group g in 0..1, 6 heads each).
Each core computes, for its (b, g):
  qT/kT = (Wq_g @ x_b^T + bq_g)  laid out [d_on_partitions, L]
  v     = x_b @ Wv_g^T + bv_g    natural layout [L, 384], interleaved with a
          ones column per head ([L, 6, 65]) so att@v also yields the softmax
          denominator for free.
  scores are computed *transposed* (sT[j, i]) so that softmax needs no
  transpose at all: exp on ACT, multiplicative 0/1 mask (host-built, bf16),
  att@v via lhsT=v (natural layout), denominator broadcast across partitions
  via a K=1 PE matmul, then the partial output projection with Wp[:, g]^T.
Host sums the two per-batch partials and adds bp.
"""

import sys

sys.path.insert(0, "/opt/trn_rl_repo")

from contextlib import ExitStack

import ml_dtypes
import numpy as np

import concourse.bacc as bacc
import concourse.bass as bass
import concourse.mybir as mybir
import concourse.tile as tile
from concourse.bass_utils import run_bass_kernel_spmd

# ---- problem constants (hardcoded per spec) ----
B = 4
T = 512
N = 8
C = 768
NHEAD = 12
L = 3 * T + 4 * N  # 1568
P = 128
G = C // 2  # 384 channels per head-group
NH = 6  # heads per core
D = 64  # head dim
ET = C // P  # 6 e-tiles (contraction of x @ W)
CT = G // P  # 3 c-tiles of the group's channels
NJT = (L + P - 1) // P  # 13 j tiles (12x128 + 32)
JPAD = NJT * P  # 1664
I_CHUNKS = [(0, 512), (512, 512), (1024, 512), (1536, 32)]
SCALE = 1.0 / 8.0  # 1/sqrt(64)

F32 = mybir.dt.float32
BF16 = mybir.dt.bfloat16
F16 = mybir.dt.float16

_NC = None  # cached compiled Bass program


def _jl(jt):
    return P if jt < NJT - 1 else L - (NJT - 1) * P  # 128 or 32


def _score_intervals(jt):
    """i-ranges (start, len) that can attend any column in j-tile jt.
    Derived from the cross-conditional mask block structure."""
    if jt <= 3:
        j0 = jt * P
        return [(j0, 512 - j0), (512 + j0, 512 - j0), (1024 + j0, 512 - j0), (1536, 32)]
    if jt <= 11:
        f0 = (jt % 4) * P
        return [(512 + f0, 512 - f0), (1024 + f0, 512 - f0), (1536, 32)]
    return [(512, 512), (1024, 512), (1536, 32)]


def _ich_of(a):
    return 3 if a == 1536 else a // 512


_ATTV_LAST = {0: 3, 1: NJT - 1, 2: NJT - 1, 3: NJT - 1}  # last jt per ich


def _build_program():
    nc = bacc.Bacc("TRN2", target_bir_lowering=False, debug=False)

    xT_d = nc.dram_tensor("xT", [C, L], F16, kind="ExternalInput")
    wq_d = nc.dram_tensor("wqT", [C, G], F16, kind="ExternalInput")
    wk_d = nc.dram_tensor("wkT", [C, G], F16, kind="ExternalInput")
    wv_d = nc.dram_tensor("wvT", [C, G], F16, kind="ExternalInput")
    wp_d = nc.dram_tensor("wpT", [G, C], F16, kind="ExternalInput")
    bq_d = nc.dram_tensor("bqP", [P, CT], F32, kind="ExternalInput")
    bk_d = nc.dram_tensor("bkP", [P, CT], F32, kind="ExternalInput")
    bv_d = nc.dram_tensor("bvB", [P, G], F32, kind="ExternalInput")
    maskd_d = nc.dram_tensor("maskD", [P, 2, P], F16, kind="ExternalInput")
    maskt_d = nc.dram_tensor("maskTxt", [32, 1024], F16, kind="ExternalInput")
    out_d = nc.dram_tensor("out_part", [L, C], F32, kind="ExternalOutput")

    with tile.TileContext(nc) as tc, ExitStack() as big:
        persist = big.enter_context(tc.tile_pool(name="persist", bufs=1))

        # persistent SBUF tensors
        qT = persist.tile([P, CT, L], F16, name="qT")
        kT = persist.tile([P, CT, L], F16, name="kT")
        v_ones = persist.tile([P, NJT, NH, D + 1], F16, name="v_ones")
        maskD = persist.tile([P, 2, P], F16, name="maskD_sb")
        maskTx = persist.tile([32, 1024], F16, name="maskTx_sb")
        yT = persist.tile([P, CT, L], F16, name="yT")
        wp_sb = persist.tile([P, CT, C], F16, name="wp_sb")
        ones64 = persist.tile([1, D], F32, name="ones64")
        bv_sb = persist.tile([P, G], F32, name="bv_sb")

        nc.sync.dma_start(maskD[:], maskd_d[:])
        nc.sync.dma_start(maskTx[:], maskt_d[:])
        nc.sync.dma_start(wp_sb[:], wp_d.rearrange("(ct p) n -> p ct n", p=P))
        nc.sync.dma_start(bv_sb[:], bv_d[:])
        nc.gpsimd.memset(ones64[:], 1.0)
        nc.gpsimd.memset(v_ones[:], 1.0)

        # ---------- Phase A: projections ----------
        with (
            tc.tile_pool(name="phA", bufs=1) as phA,
            tc.tile_pool(name="psA", bufs=2, space="PSUM") as psA,
        ):
            xT = phA.tile([P, ET, L], F16, name="xT_sb")
            wq_sb = phA.tile([P, ET, G], F16, name="wq_sb")
            wk_sb = phA.tile([P, ET, G], F16, name="wk_sb")
            wv_sb = phA.tile([P, ET, G], F16, name="wv_sb")
            bq_sb = phA.tile([P, CT], F32, name="bq_sb")
            bk_sb = phA.tile([P, CT], F32, name="bk_sb")

            nc.sync.dma_start(xT[:], xT_d.rearrange("(et p) i -> p et i", p=P))
            nc.sync.dma_start(wq_sb[:], wq_d.rearrange("(et p) m -> p et m", p=P))
            nc.sync.dma_start(wk_sb[:], wk_d.rearrange("(et p) m -> p et m", p=P))
            nc.sync.dma_start(wv_sb[:], wv_d.rearrange("(et p) m -> p et m", p=P))
            nc.sync.dma_start(bq_sb[:], bq_d[:])
            nc.sync.dma_start(bk_sb[:], bk_d[:])

            # qT / kT: out[c_tile, i] accumulated over e tiles
            for dst, w_sb, b_sb in ((qT, wq_sb, bq_sb), (kT, wk_sb, bk_sb)):
                for ct in range(CT):
                    for i0, ilen in I_CHUNKS:
                        ps = psA.tile([P, 512], F32, name="ps_qk", tag="ps_qk")
                        for et in range(ET):
                            nc.tensor.matmul(
                                ps[:, :ilen],
                                w_sb[:, et, ct * P : (ct + 1) * P],
                                xT[:, et, i0 : i0 + ilen],
                                start=(et == 0),
                                stop=(et == ET - 1),
                            )
                        nc.vector.tensor_scalar(
                            dst[:, ct, i0 : i0 + ilen],
                            ps[:, :ilen],
                            b_sb[:, ct : ct + 1],
                            None,
                            mybir.AluOpType.add,
                        )

            # v natural layout [i, 384] + bias, into the 65-strided bf16 buffer
            for it in range(NJT):
                il = _jl(it)
                ps = psA.tile([P, G], F32, name="ps_v", tag="ps_v")
                for et in range(ET):
                    nc.tensor.matmul(
                        ps[:il, :],
                        xT[:, et, it * P : it * P + il],
                        wv_sb[:, et, :],
                        start=(et == 0),
                        stop=(et == ET - 1),
                    )
                nc.vector.tensor_tensor(
                    v_ones[:il, it, :, 0:D],
                    ps[:il, :].rearrange("p (h d) -> p h d", h=NH),
                    bv_sb[:il, :].rearrange("p (h d) -> p h d", h=NH),
                    mybir.AluOpType.add,
                )

        # ---------- Phase B: attention per head (jt-major, block-skipped) ----------
        with (
            tc.tile_pool(name="phB", bufs=1) as phB,
            tc.tile_pool(name="psS", bufs=3, space="PSUM") as psS,
            tc.tile_pool(name="psY", bufs=5, space="PSUM") as psY,
        ):
            for h in range(NH):
                pof = D * (h % 2)
                ct = h // 2
                ps_y = [
                    psY.tile([D + 1, 512], F32, name=f"ps_y{ich}", tag="ps_y")
                    for ich in range(4)
                ]
                started = [False] * 4
                for jt in range(NJT):
                    jl = _jl(jt)
                    for k, (a, ln) in enumerate(_score_intervals(jt)):
                        ps_s = psS.tile([P, 512], F32, name="ps_s", tag="ps_s")
                        nc.tensor.matmul(
                            ps_s[:jl, :ln],
                            kT[pof : pof + D, ct, jt * P : jt * P + jl],
                            qT[pof : pof + D, ct, a : a + ln],
                            start=True,
                            stop=True,
                        )
                        pt = phB.tile([P, 512], F16, name="pT", tag="pT", bufs=14)
                        nc.scalar.activation(
                            pt[:jl, :ln],
                            ps_s[:jl, :ln],
                            mybir.ActivationFunctionType.Exp,
                            bias=0.0,
                            scale=SCALE,
                        )
                        # selective masking: diagonal 128-block or text-column strip
                        if jt <= 11 and k < (3 if jt <= 3 else 2):
                            # tril (T1) for U-cols always, and torso-rows @ L-cols;
                            # strictly-lower (T2) elsewhere (see reference mask)
                            didx = 0 if (jt <= 3 or (jt <= 7 and k == 1)) else 1
                            nc.vector.tensor_tensor(
                                pt[:jl, 0:P],
                                pt[:jl, 0:P],
                                maskD[:jl, didx, :],
                                mybir.AluOpType.mult,
                            )
                        elif jt == 12 and k < 2:
                            nc.vector.tensor_tensor(
                                pt[:jl, :ln],
                                pt[:jl, :ln],
                                maskTx[:jl, a - 512 : a - 512 + ln],
                                mybir.AluOpType.mult,
                            )
                        ich = _ich_of(a)
                        off = a - (0, 512, 1024, 1536)[ich]
                        nc.tensor.matmul(
                            ps_y[ich][:, off : off + ln],
                            v_ones[:jl, jt, h, :],
                            pt[:jl, :ln],
                            start=not started[ich],
                            stop=(jt == _ATTV_LAST[ich]),
                            skip_group_check=True,
                        )
                        started[ich] = True

                for ich, (i0, ilen) in enumerate(I_CHUNKS):
                    den = phB.tile([1, 512], F32, name="den", tag="den", bufs=4)
                    nc.scalar.activation(
                        den[0:1, :ilen],
                        ps_y[ich][D : D + 1, :ilen],
                        mybir.ActivationFunctionType.Copy,
                    )
                    ps_bc = psS.tile([D, 512], F32, name="ps_bc", tag="ps_s")
                    nc.tensor.matmul(
                        ps_bc[:, :ilen],
                        ones64[0:1, :],
                        den[0:1, :ilen],
                        start=True,
                        stop=True,
                    )
                    rc = phB.tile([D, 512], F32, name="rc", tag="rc", bufs=4)
                    nc.vector.reciprocal_approx_fast(out=rc[:, :ilen], in_=ps_bc[:, :ilen])
                    nc.vector.tensor_tensor(
                        yT[pof : pof + D, ct, i0 : i0 + ilen],
                        ps_y[ich][0:D, :ilen],
                        rc[:, :ilen],
                        mybir.AluOpType.mult,
                    )

        # ---------- Phase C: output projection (partial) ----------
        with (
            tc.tile_pool(name="phC", bufs=3) as phC,
            tc.tile_pool(name="psC", bufs=2, space="PSUM") as psC,
        ):
            for it in range(NJT):
                il = _jl(it)
                o_sb = phC.tile([P, C], F32, name="o_sb", tag="o_sb")
                for nch in range(2):
                    ps_o = psC.tile([P, 384], F32, name="ps_o", tag="ps_o")
                    for kt in range(CT):
                        nc.tensor.matmul(
                            ps_o[:il, :],
                            yT[:, kt, it * P : it * P + il],
                            wp_sb[:, kt, nch * 384 : (nch + 1) * 384],
                            start=(kt == 0),
                            stop=(kt == CT - 1),
                        )
                    nc.any.tensor_copy(o_sb[:il, nch * 384 : (nch + 1) * 384], ps_o[:il, :])
                nc.sync.dma_start(out_d[it * P : it * P + il, :], o_sb[:il, :])

    nc.compile()
    return nc


def _build_mask_np(seg_starts, seg_ends):
    """True = masked. Mirrors reference._build_mask in numpy."""
    ML = 3 * T
    tril = np.tril(np.ones((T, T), dtype=bool))
    sl = np.tril(np.ones((T, T), dtype=bool), -1)
    m = np.zeros((L, L), dtype=bool)
    m[:ML, :ML] = True
    m[0:T, 0:T] = ~tril
    m[T : 2 * T, 0:T] = ~tril
    m[T : 2 * T, T : 2 * T] = ~sl
    m[T : 2 * T, 2 * T : 3 * T] = ~sl
    m[2 * T : 3 * T, 0:T] = ~tril
    m[2 * T : 3 * T, T : 2 * T] = ~tril
    m[2 * T : 3 * T, 2 * T : 3 * T] = ~sl
    m[:ML, ML:] = True
    frames = np.arange(T)[None, :, None]
    allowed = (frames >= seg_starts[:, None, :]) & (frames < seg_ends[:, None, :])
    mask = np.broadcast_to(m[None], (B, L, L)).copy()
    for row0, col_blocks in ((T, (0, 2, 3)), (2 * T, (1, 2, 3))):
        for j in col_blocks:
            c0 = ML + j * N
            mask[:, row0 : row0 + T, c0 : c0 + N] &= ~allowed
    return mask


def get_nc():
    global _NC
    if _NC is None:
        _NC = _build_program()
    return _NC


def make_in_maps(x, Wq, bq, Wk, bk, Wv, bv, Wp, bp, seg_starts, seg_ends):
    mask = _build_mask_np(np.asarray(seg_starts), np.asarray(seg_ends))
    r = np.arange(P)
    maskD = np.empty((P, 2, P), dtype=np.float16)
    maskD[:, 0, :] = (r[:, None] <= r[None, :]).astype(np.float16)  # tril.T
    maskD[:, 1, :] = (r[:, None] < r[None, :]).astype(np.float16)  # strict
    in_maps = []
    for core in range(8):
        b, g = core // 2, core % 2
        gs = slice(g * G, (g + 1) * G)
        allowT = ~mask[b].T  # [j, i]
        maskTx = np.ascontiguousarray(
            allowT[1536:1568, 512:1536].astype(np.float16)
        )
        in_maps.append(
            {
                "xT": np.ascontiguousarray(x[b].T).astype(np.float16),
                "wqT": np.ascontiguousarray(Wq[gs, :].T).astype(np.float16),
                "wkT": np.ascontiguousarray(Wk[gs, :].T).astype(np.float16),
                "wvT": np.ascontiguousarray(Wv[gs, :].T).astype(np.float16),
                "wpT": np.ascontiguousarray(Wp[:, gs].T).astype(np.float16),
                "bqP": np.ascontiguousarray(bq[gs].reshape(CT, P).T),
                "bkP": np.ascontiguousarray(bk[gs].reshape(CT, P).T),
                "bvB": np.broadcast_to(bv[gs], (P, G)).copy(),
                "maskD": maskD,
                "maskTxt": maskTx,
            }
        )
    return in_maps


def kernel(x, Wq, bq, Wk, bk, Wv, bv, Wp, bp, seg_starts, seg_ends, T_motion=None,
           N=None, _trace=False, **_unused):
    x = np.asarray(x, np.float32)
    args = [np.asarray(a, np.float32) for a in (Wq, bq, Wk, bk, Wv, bv, Wp, bp)]
    Wq, bq, Wk, bk, Wv, bv, Wp, bp = args
    nc = get_nc()
    in_maps = make_in_maps(x, Wq, bq, Wk, bk, Wv, bv, Wp, bp, seg_starts, seg_ends)
    res = run_bass_kernel_spmd(nc, in_maps, core_ids=list(range(8)), trace=_trace)
    parts = [r["out_part"] for r in res.results]
    y = np.empty((B, L, C), np.float32)
    for b in range(B):
        y[b] = parts[2 * b] + parts[2 * b + 1] + bp
    if _trace:
        kernel.last_results = res
    return y


# revision 12
# speedup vs baseline: 2.9488x; 1.0676x over previous
"""Cross-conditional GPT2 sparse attention block on 8 Trainium2 NeuronCores.

Sharding: core = (batch b in 0..3) x (head-group g in 0..1, 6 heads each).
Each core computes, for its (b, g):
  qT/kT = (Wq_g @ x_b^T + bq_g)  laid out [d_on_partitions, L]
  v     = x_b @ Wv_g^T + bv_g    natural layout [L, 384], interleaved with a
          ones column per head ([L, 6, 65]) so att@v also yields the softmax
          denominator for free.
  scores are computed *transposed* (sT[j, i]) so that softmax needs no
  transpose at all: exp on ACT, multiplicative 0/1 mask (host-built, bf16),
  att@v via lhsT=v (natural layout), denominator broadcast across partitions
  via a K=1 PE matmul, then the partial output projection with Wp[:, g]^T.
Host sums the two per-batch partials and adds bp.
"""

import sys

sys.path.insert(0, "/opt/trn_rl_repo")

from contextlib import ExitStack

import ml_dtypes
import numpy as np

import concourse.bacc as bacc
import concourse.bass as bass
import concourse.mybir as mybir
import concourse.tile as tile
from concourse.bass_utils import run_bass_kernel_spmd

# ---- problem constants (hardcoded per spec) ----
B = 4
T = 512
N = 8
C = 768
NHEAD = 12
L = 3 * T + 4 * N  # 1568
P = 128
G = C // 2  # 384 channels per head-group
NH = 6  # heads per core
D = 64  # head dim
ET = C // P  # 6 e-tiles (contraction of x @ W)
CT = G // P  # 3 c-tiles of the group's channels
NJT = (L + P - 1) // P  # 13 j tiles (12x128 + 32)
JPAD = NJT * P  # 1664
I_CHUNKS = [(0, 512), (512, 512), (1024, 512), (1536, 32)]
SCALE = 1.0 / 8.0  # 1/sqrt(64)

F32 = mybir.dt.float32
BF16 = mybir.dt.bfloat16
F16 = mybir.dt.float16

_NC = None  # cached compiled Bass program


def _jl(jt):
    return P if jt < NJT - 1 else L - (NJT - 1) * P  # 128 or 32


def _score_intervals(jt):
    """i-ranges (start, len) that can attend any column in j-tile jt.
    Derived from the cross-conditional mask block structure. The text-row
    strip [1536,1568) is merged into the preceding torso interval whenever
    the combined length fits one PSUM bank (<=512)."""
    if jt <= 3:
        j0 = jt * P
        iv = [(j0, 512 - j0), (512 + j0, 512 - j0), (1024 + j0, 512 - j0), (1536, 32)]
    elif jt <= 11:
        f0 = (jt % 4) * P
        iv = [(512 + f0, 512 - f0), (1024 + f0, 512 - f0), (1536, 32)]
    else:
        iv = [(512, 512), (1024, 512), (1536, 32)]
    if len(iv) >= 2 and iv[-2][0] + iv[-2][1] == 1536 and iv[-2][1] + 32 <= 512:
        iv = iv[:-2] + [(iv[-2][0], iv[-2][1] + 32)]
    return iv


def _ich_of(a):
    return 3 if a == 1536 else a // 512


_ATTV_LAST = {0: 3, 1: NJT - 1, 2: NJT - 1, 3: NJT - 1}  # last jt per ich


def _build_program():
    nc = bacc.Bacc("TRN2", target_bir_lowering=False, debug=False)

    xT_d = nc.dram_tensor("xT", [C, L], F16, kind="ExternalInput")
    wq_d = nc.dram_tensor("wqT", [C, G], F16, kind="ExternalInput")
    wk_d = nc.dram_tensor("wkT", [C, G], F16, kind="ExternalInput")
    wv_d = nc.dram_tensor("wvT", [C, G], F16, kind="ExternalInput")
    wp_d = nc.dram_tensor("wpT", [G, C], F16, kind="ExternalInput")
    bq_d = nc.dram_tensor("bqP", [P, CT], F32, kind="ExternalInput")
    bk_d = nc.dram_tensor("bkP", [P, CT], F32, kind="ExternalInput")
    bv_d = nc.dram_tensor("bvB", [P, G], F32, kind="ExternalInput")
    maskd_d = nc.dram_tensor("maskD", [P, 2, P], F16, kind="ExternalInput")
    maskt_d = nc.dram_tensor("maskTxt", [32, 1024], F16, kind="ExternalInput")
    out_d = nc.dram_tensor("out_part", [L, C], F32, kind="ExternalOutput")

    with tile.TileContext(nc) as tc, ExitStack() as big:
        persist = big.enter_context(tc.tile_pool(name="persist", bufs=1))

        # persistent SBUF tensors
        qT = persist.tile([P, CT, L], F16, name="qT")
        kT = persist.tile([P, CT, L], F16, name="kT")
        v_ones = persist.tile([P, NJT, NH, D + 1], F16, name="v_ones")
        maskD = persist.tile([P, 2, P], F16, name="maskD_sb")
        maskTx = persist.tile([32, 1024], F16, name="maskTx_sb")
        yT = persist.tile([P, CT, L], F16, name="yT")
        wp_sb = persist.tile([P, CT, C], F16, name="wp_sb")
        ones64 = persist.tile([1, D], F16, name="ones64")
        bv_sb = persist.tile([P, G], F32, name="bv_sb")

        nc.sync.dma_start(maskD[:], maskd_d[:])
        nc.sync.dma_start(maskTx[:], maskt_d[:])
        nc.sync.dma_start(wp_sb[:], wp_d.rearrange("(ct p) n -> p ct n", p=P))
        nc.sync.dma_start(bv_sb[:], bv_d[:])
        nc.gpsimd.memset(ones64[:], 1.0)
        nc.gpsimd.memset(v_ones[:], 1.0)

        # ---------- Phase A: projections ----------
        with (
            tc.tile_pool(name="phA", bufs=1) as phA,
            tc.tile_pool(name="psA", bufs=2, space="PSUM") as psA,
        ):
            xT = phA.tile([P, ET, L], F16, name="xT_sb")
            wq_sb = phA.tile([P, ET, G], F16, name="wq_sb")
            wk_sb = phA.tile([P, ET, G], F16, name="wk_sb")
            wv_sb = phA.tile([P, ET, G], F16, name="wv_sb")
            bq_sb = phA.tile([P, CT], F32, name="bq_sb")
            bk_sb = phA.tile([P, CT], F32, name="bk_sb")

            nc.sync.dma_start(xT[:], xT_d.rearrange("(et p) i -> p et i", p=P))
            nc.sync.dma_start(wq_sb[:], wq_d.rearrange("(et p) m -> p et m", p=P))
            nc.sync.dma_start(wk_sb[:], wk_d.rearrange("(et p) m -> p et m", p=P))
            nc.sync.dma_start(wv_sb[:], wv_d.rearrange("(et p) m -> p et m", p=P))
            nc.sync.dma_start(bq_sb[:], bq_d[:])
            nc.sync.dma_start(bk_sb[:], bk_d[:])

            # qT / kT: out[c_tile, i] accumulated over e tiles
            for dst, w_sb, b_sb in ((qT, wq_sb, bq_sb), (kT, wk_sb, bk_sb)):
                for ct in range(CT):
                    for i0, ilen in I_CHUNKS:
                        ps = psA.tile([P, 512], F32, name="ps_qk", tag="ps_qk")
                        for et in range(ET):
                            nc.tensor.matmul(
                                ps[:, :ilen],
                                w_sb[:, et, ct * P : (ct + 1) * P],
                                xT[:, et, i0 : i0 + ilen],
                                start=(et == 0),
                                stop=(et == ET - 1),
                            )
                        nc.vector.tensor_scalar(
                            dst[:, ct, i0 : i0 + ilen],
                            ps[:, :ilen],
                            b_sb[:, ct : ct + 1],
                            None,
                            mybir.AluOpType.add,
                        )

            # v natural layout [i, 384] + bias, into the 65-strided bf16 buffer
            for it in range(NJT):
                il = _jl(it)
                ps = psA.tile([P, G], F32, name="ps_v", tag="ps_v")
                for et in range(ET):
                    nc.tensor.matmul(
                        ps[:il, :],
                        xT[:, et, it * P : it * P + il],
                        wv_sb[:, et, :],
                        start=(et == 0),
                        stop=(et == ET - 1),
                    )
                nc.vector.tensor_tensor(
                    v_ones[:il, it, :, 0:D],
                    ps[:il, :].rearrange("p (h d) -> p h d", h=NH),
                    bv_sb[:il, :].rearrange("p (h d) -> p h d", h=NH),
                    mybir.AluOpType.add,
                )

        # ---------- Phase B: attention per head (jt-major, block-skipped) ----------
        with (
            tc.tile_pool(name="phB", bufs=1) as phB,
            tc.tile_pool(name="psS", bufs=3, space="PSUM") as psS,
            tc.tile_pool(name="psY", bufs=5, space="PSUM") as psY,
        ):
            for h in range(NH):
                pof = D * (h % 2)
                ct = h // 2
                ps_y = [
                    psY.tile([D + 1, 512], F32, name=f"ps_y{ich}", tag="ps_y")
                    for ich in range(4)
                ]
                started = [False] * 4
                for jt in range(NJT):
                    jl = _jl(jt)
                    ivs = _score_intervals(jt)
                    pts = []
                    for k, (a, ln) in enumerate(ivs):
                        ps_s = psS.tile([P, 512], F32, name="ps_s", tag="ps_s")
                        nc.tensor.matmul(
                            ps_s[:jl, :ln],
                            kT[pof : pof + D, ct, jt * P : jt * P + jl],
                            qT[pof : pof + D, ct, a : a + ln],
                            start=True,
                            stop=True,
                        )
                        pt = phB.tile([P, 512], F16, name="pT", tag="pT", bufs=14)
                        nc.scalar.activation(
                            pt[:jl, :ln],
                            ps_s[:jl, :ln],
                            mybir.ActivationFunctionType.Exp,
                            bias=0.0,
                            scale=SCALE,
                        )
                        # selective masking: diagonal 128-block or text-column strip
                        if jt <= 11 and k < (3 if jt <= 3 else 2):
                            # tril (T1) for U-cols always, and torso-rows @ L-cols;
                            # strictly-lower (T2) elsewhere (see reference mask)
                            didx = 0 if (jt <= 3 or (jt <= 7 and k == 1)) else 1
                            nc.vector.tensor_tensor(
                                pt[:jl, 0:P],
                                pt[:jl, 0:P],
                                maskD[:jl, didx, :],
                                mybir.AluOpType.mult,
                            )
                        elif jt == 12 and k < 2:
                            nc.vector.tensor_tensor(
                                pt[:jl, :ln],
                                pt[:jl, :ln],
                                maskTx[:jl, a - 512 : a - 512 + ln],
                                mybir.AluOpType.mult,
                            )
                        pts.append(pt)

                    # att@v: same stationary v tile for all chunks of this jt
                    for k, (a, ln) in enumerate(ivs):
                        parts = [(a, ln, 0)]
                        if a < 1536 < a + ln:  # merged torso+text interval
                            parts = [(a, 1536 - a, 0), (1536, a + ln - 1536, 1536 - a)]
                        for pa, pl, poff in parts:
                            ich = _ich_of(pa)
                            off = pa - (0, 512, 1024, 1536)[ich]
                            nc.tensor.matmul(
                                ps_y[ich][:, off : off + pl],
                                v_ones[:jl, jt, h, :],
                                pts[k][:jl, poff : poff + pl],
                                start=not started[ich],
                                stop=(jt == _ATTV_LAST[ich]),
                                skip_group_check=True,
                            )
                            started[ich] = True

                for ich, (i0, ilen) in enumerate(I_CHUNKS):
                    den = phB.tile([1, 512], F16, name="den", tag="den", bufs=4)
                    nc.vector.tensor_copy(den[0:1, :ilen], ps_y[ich][D : D + 1, :ilen])
                    ps_bc = psS.tile([D, 512], F32, name="ps_bc", tag="ps_s")
                    nc.tensor.matmul(
                        ps_bc[:, :ilen],
                        ones64[0:1, :],
                        den[0:1, :ilen],
                        start=True,
                        stop=True,
                    )
                    rc = phB.tile([D, 512], F32, name="rc", tag="rc", bufs=4)
                    nc.vector.reciprocal_approx_fast(out=rc[:, :ilen], in_=ps_bc[:, :ilen])
                    nc.vector.tensor_tensor(
                        yT[pof : pof + D, ct, i0 : i0 + ilen],
                        ps_y[ich][0:D, :ilen],
                        rc[:, :ilen],
                        mybir.AluOpType.mult,
                    )

        # ---------- Phase C: output projection (partial) ----------
        with (
            tc.tile_pool(name="phC", bufs=3) as phC,
            tc.tile_pool(name="psC", bufs=2, space="PSUM") as psC,
        ):
            for it in range(NJT):
                il = _jl(it)
                o_sb = phC.tile([P, C], F32, name="o_sb", tag="o_sb")
                for nch in range(2):
                    ps_o = psC.tile([P, 384], F32, name="ps_o", tag="ps_o")
                    for kt in range(CT):
                        nc.tensor.matmul(
                            ps_o[:il, :],
                            yT[:, kt, it * P : it * P + il],
                            wp_sb[:, kt, nch * 384 : (nch + 1) * 384],
                            start=(kt == 0),
                            stop=(kt == CT - 1),
                        )
                    nc.any.tensor_copy(o_sb[:il, nch * 384 : (nch + 1) * 384], ps_o[:il, :])
                nc.sync.dma_start(out_d[it * P : it * P + il, :], o_sb[:il, :])

    nc.compile()
    return nc


def _build_mask_np(seg_starts, seg_ends):
    """True = masked. Mirrors reference._build_mask in numpy."""
    ML = 3 * T
    tril = np.tril(np.ones((T, T), dtype=bool))
    sl = np.tril(np.ones((T, T), dtype=bool), -1)
    m = np.zeros((L, L), dtype=bool)
    m[:ML, :ML] = True
    m[0:T, 0:T] = ~tril
    m[T : 2 * T, 0:T] = ~tril
    m[T : 2 * T, T : 2 * T] = ~sl
    m[T : 2 * T, 2 * T : 3 * T] = ~sl
    m[2 * T : 3 * T, 0:T] = ~tril
    m[2 * T : 3 * T, T : 2 * T] = ~tril
    m[2 * T : 3 * T, 2 * T : 3 * T] = ~sl
    m[:ML, ML:] = True
    frames = np.arange(T)[None, :, None]
    allowed = (frames >= seg_starts[:, None, :]) & (frames < seg_ends[:, None, :])
    mask = np.broadcast_to(m[None], (B, L, L)).copy()
    for row0, col_blocks in ((T, (0, 2, 3)), (2 * T, (1, 2, 3))):
        for j in col_blocks:
            c0 = ML + j * N
            mask[:, row0 : row0 + T, c0 : c0 + N] &= ~allowed
    return mask


def get_nc():
    global _NC
    if _NC is None:
        _NC = _build_program()
    return _NC


def make_in_maps(x, Wq, bq, Wk, bk, Wv, bv, Wp, bp, seg_starts, seg_ends):
    mask = _build_mask_np(np.asarray(seg_starts), np.asarray(seg_ends))
    r = np.arange(P)
    maskD = np.empty((P, 2, P), dtype=np.float16)
    maskD[:, 0, :] = (r[:, None] <= r[None, :]).astype(np.float16)  # tril.T
    maskD[:, 1, :] = (r[:, None] < r[None, :]).astype(np.float16)  # strict
    in_maps = []
    for core in range(8):
        b, g = core // 2, core % 2
        gs = slice(g * G, (g + 1) * G)
        allowT = ~mask[b].T  # [j, i]
        maskTx = np.ascontiguousarray(
            allowT[1536:1568, 512:1536].astype(np.float16)
        )
        in_maps.append(
            {
                "xT": np.ascontiguousarray(x[b].T).astype(np.float16),
                "wqT": np.ascontiguousarray(Wq[gs, :].T).astype(np.float16),
                "wkT": np.ascontiguousarray(Wk[gs, :].T).astype(np.float16),
                "wvT": np.ascontiguousarray(Wv[gs, :].T).astype(np.float16),
                "wpT": np.ascontiguousarray(Wp[:, gs].T).astype(np.float16),
                "bqP": np.ascontiguousarray(bq[gs].reshape(CT, P).T),
                "bkP": np.ascontiguousarray(bk[gs].reshape(CT, P).T),
                "bvB": np.broadcast_to(bv[gs], (P, G)).copy(),
                "maskD": maskD,
                "maskTxt": maskTx,
            }
        )
    return in_maps


def kernel(x, Wq, bq, Wk, bk, Wv, bv, Wp, bp, seg_starts, seg_ends, T_motion=None,
           N=None, _trace=False, **_unused):
    x = np.asarray(x, np.float32)
    args = [np.asarray(a, np.float32) for a in (Wq, bq, Wk, bk, Wv, bv, Wp, bp)]
    Wq, bq, Wk, bk, Wv, bv, Wp, bp = args
    nc = get_nc()
    in_maps = make_in_maps(x, Wq, bq, Wk, bk, Wv, bv, Wp, bp, seg_starts, seg_ends)
    res = run_bass_kernel_spmd(nc, in_maps, core_ids=list(range(8)), trace=_trace)
    parts = [r["out_part"] for r in res.results]
    y = np.empty((B, L, C), np.float32)
    for b in range(B):
        y[b] = parts[2 * b] + parts[2 * b + 1] + bp
    if _trace:
        kernel.last_results = res
    return y


# revision 13
# speedup vs baseline: 3.3336x; 1.1305x over previous
"""Cross-conditional GPT2 sparse attention block on 8 Trainium2 NeuronCores.

Sharding: core = (batch b in 0..3) x (head-group g in 0..1, 6 heads each).
Each core computes, for its (b, g):
  qT/kT = (Wq_g @ x_b^T + bq_g)  laid out [d_on_partitions, L]
  v     = x_b @ Wv_g^T + bv_g    natural layout [L, 384], interleaved with a
          ones column per head ([L, 6, 65]) so att@v also yields the softmax
          denominator for free.
  scores are computed *transposed* (sT[j, i]) so that softmax needs no
  transpose at all: exp on ACT, multiplicative 0/1 mask (host-built, bf16),
  att@v via lhsT=v (natural layout), denominator broadcast across partitions
  via a K=1 PE matmul, then the partial output projection with Wp[:, g]^T.
Host sums the two per-batch partials and adds bp.
"""

import sys

sys.path.insert(0, "/opt/trn_rl_repo")

from contextlib import ExitStack

import ml_dtypes
import numpy as np

import concourse.bacc as bacc
import concourse.bass as bass
import concourse.mybir as mybir
import concourse.tile as tile
from concourse.bass_utils import run_bass_kernel_spmd

# ---- problem constants (hardcoded per spec) ----
B = 4
T = 512
N = 8
C = 768
NHEAD = 12
L = 3 * T + 4 * N  # 1568
P = 128
G = C // 2  # 384 channels per head-group
NH = 6  # heads per core
D = 64  # head dim
ET = C // P  # 6 e-tiles (contraction of x @ W)
CT = G // P  # 3 c-tiles of the group's channels
NJT = (L + P - 1) // P  # 13 j tiles (12x128 + 32)
JPAD = NJT * P  # 1664
I_CHUNKS = [(0, 512), (512, 512), (1024, 512), (1536, 32)]
SCALE = 1.0 / 8.0  # 1/sqrt(64)

F32 = mybir.dt.float32
BF16 = mybir.dt.bfloat16
F16 = mybir.dt.float16

_NC = None  # cached compiled Bass program


def _jl(jt):
    return P if jt < NJT - 1 else L - (NJT - 1) * P  # 128 or 32


def _score_intervals(jt):
    """i-ranges (start, len) that can attend any column in j-tile jt.
    Derived from the cross-conditional mask block structure. The text-row
    strip [1536,1568) is merged into the preceding torso interval whenever
    the combined length fits one PSUM bank (<=512)."""
    if jt <= 3:
        j0 = jt * P
        iv = [(j0, 512 - j0), (512 + j0, 512 - j0), (1024 + j0, 512 - j0), (1536, 32)]
    elif jt <= 11:
        f0 = (jt % 4) * P
        iv = [(512 + f0, 512 - f0), (1024 + f0, 512 - f0), (1536, 32)]
    else:
        iv = [(512, 512), (1024, 512), (1536, 32)]
    if len(iv) >= 2 and iv[-2][0] + iv[-2][1] == 1536 and iv[-2][1] + 32 <= 512:
        iv = iv[:-2] + [(iv[-2][0], iv[-2][1] + 32)]
    return iv


def _ich_of(a):
    return 3 if a == 1536 else a // 512


_ATTV_LAST = {0: 3, 1: NJT - 1, 2: NJT - 1, 3: NJT - 1}  # last jt per ich


def _build_program():
    nc = bacc.Bacc("TRN2", target_bir_lowering=False, debug=False)

    xT_d = nc.dram_tensor("xT", [C, L], F16, kind="ExternalInput")
    wq_d = nc.dram_tensor("wqT", [C, G], F16, kind="ExternalInput")
    wk_d = nc.dram_tensor("wkT", [C, G], F16, kind="ExternalInput")
    wv_d = nc.dram_tensor("wvT", [C, G], F16, kind="ExternalInput")
    wp_d = nc.dram_tensor("wpT", [G, C], F16, kind="ExternalInput")
    bq_d = nc.dram_tensor("bqP", [P, CT], F32, kind="ExternalInput")
    bk_d = nc.dram_tensor("bkP", [P, CT], F32, kind="ExternalInput")
    bv_d = nc.dram_tensor("bvB", [P, G], F32, kind="ExternalInput")
    maskd_d = nc.dram_tensor("maskD", [P, 2, P], F16, kind="ExternalInput")
    maskt_d = nc.dram_tensor("maskTxt", [32, 1024], F16, kind="ExternalInput")
    out_d = nc.dram_tensor("out_part", [L, C], F32, kind="ExternalOutput")

    with tile.TileContext(nc) as tc, ExitStack() as big:
        persist = big.enter_context(tc.tile_pool(name="persist", bufs=1))

        # persistent SBUF tensors
        qT = persist.tile([P, CT, L], F16, name="qT")
        kT = persist.tile([P, CT, L], F16, name="kT")
        v_ones = persist.tile([P, NJT, NH, D + 1], F16, name="v_ones")
        maskD = persist.tile([P, 2, P], F16, name="maskD_sb")
        maskTx = persist.tile([32, 1024], F16, name="maskTx_sb")
        yT = persist.tile([P, CT, L], F16, name="yT")
        wp_sb = persist.tile([P, CT, C], F16, name="wp_sb")
        ones64 = persist.tile([1, D], F16, name="ones64")
        bv_sb = persist.tile([P, G], F32, name="bv_sb")

        nc.sync.dma_start(maskD[:], maskd_d[:])
        nc.sync.dma_start(maskTx[:], maskt_d[:])
        nc.sync.dma_start(wp_sb[:], wp_d.rearrange("(ct p) n -> p ct n", p=P))
        nc.sync.dma_start(bv_sb[:], bv_d[:])
        nc.gpsimd.memset(ones64[:], 1.0)
        nc.gpsimd.memset(v_ones[:], 1.0)

        # ---------- Phase A: projections ----------
        with (
            tc.tile_pool(name="phA", bufs=1) as phA,
            tc.tile_pool(name="psA", bufs=2, space="PSUM") as psA,
        ):
            xT = phA.tile([P, ET, L], F16, name="xT_sb")
            wq_sb = phA.tile([P, ET, G], F16, name="wq_sb")
            wk_sb = phA.tile([P, ET, G], F16, name="wk_sb")
            wv_sb = phA.tile([P, ET, G], F16, name="wv_sb")
            bq_sb = phA.tile([P, CT], F32, name="bq_sb")
            bk_sb = phA.tile([P, CT], F32, name="bk_sb")

            nc.sync.dma_start(xT[:], xT_d.rearrange("(et p) i -> p et i", p=P))
            nc.sync.dma_start(wq_sb[:], wq_d.rearrange("(et p) m -> p et m", p=P))
            nc.sync.dma_start(wk_sb[:], wk_d.rearrange("(et p) m -> p et m", p=P))
            nc.sync.dma_start(wv_sb[:], wv_d.rearrange("(et p) m -> p et m", p=P))
            nc.sync.dma_start(bq_sb[:], bq_d[:])
            nc.sync.dma_start(bk_sb[:], bk_d[:])

            # qT / kT: out[c_tile, i] accumulated over e tiles
            for dst, w_sb, b_sb in ((qT, wq_sb, bq_sb), (kT, wk_sb, bk_sb)):
                for ct in range(CT):
                    for i0, ilen in I_CHUNKS:
                        ps = psA.tile([P, 512], F32, name="ps_qk", tag="ps_qk")
                        for et in range(ET):
                            nc.tensor.matmul(
                                ps[:, :ilen],
                                w_sb[:, et, ct * P : (ct + 1) * P],
                                xT[:, et, i0 : i0 + ilen],
                                start=(et == 0),
                                stop=(et == ET - 1),
                            )
                        nc.vector.tensor_scalar(
                            dst[:, ct, i0 : i0 + ilen],
                            ps[:, :ilen],
                            b_sb[:, ct : ct + 1],
                            None,
                            mybir.AluOpType.add,
                        )

            # v natural layout [i, 384] + bias, into the 65-strided bf16 buffer
            for it in range(NJT):
                il = _jl(it)
                ps = psA.tile([P, G], F32, name="ps_v", tag="ps_v")
                for et in range(ET):
                    nc.tensor.matmul(
                        ps[:il, :],
                        xT[:, et, it * P : it * P + il],
                        wv_sb[:, et, :],
                        start=(et == 0),
                        stop=(et == ET - 1),
                    )
                nc.vector.tensor_tensor(
                    v_ones[:il, it, :, 0:D],
                    ps[:il, :].rearrange("p (h d) -> p h d", h=NH),
                    bv_sb[:il, :].rearrange("p (h d) -> p h d", h=NH),
                    mybir.AluOpType.add,
                )

        # ---------- Phase B: attention per head (jt-major, block-skipped) ----------
        with (
            tc.tile_pool(name="phB", bufs=1) as phB,
            tc.tile_pool(name="psS", bufs=3, space="PSUM") as psS,
            tc.tile_pool(name="psY", bufs=4, space="PSUM") as psY,
            tc.tile_pool(name="psBC", bufs=1, space="PSUM") as psBC,
        ):
            for h in range(NH):
                pof = D * (h % 2)
                ct = h // 2
                ps_y = [
                    psY.tile([D + 1, 512], F32, name=f"ps_y{ich}", tag="ps_y")
                    for ich in range(4)
                ]
                started = [False] * 4
                for jt in range(NJT):
                    jl = _jl(jt)
                    ivs = _score_intervals(jt)
                    pts = []
                    for k, (a, ln) in enumerate(ivs):
                        ps_s = psS.tile([P, 512], F32, name="ps_s", tag="ps_s")
                        nc.tensor.matmul(
                            ps_s[:jl, :ln],
                            kT[pof : pof + D, ct, jt * P : jt * P + jl],
                            qT[pof : pof + D, ct, a : a + ln],
                            start=True,
                            stop=True,
                        )
                        pt = phB.tile([P, 512], F16, name="pT", tag="pT", bufs=20)
                        nc.scalar.activation(
                            pt[:jl, :ln],
                            ps_s[:jl, :ln],
                            mybir.ActivationFunctionType.Exp,
                            bias=0.0,
                            scale=SCALE,
                        )
                        # selective masking: diagonal 128-block or text-column strip
                        if jt <= 11 and k < (3 if jt <= 3 else 2):
                            # tril (T1) for U-cols always, and torso-rows @ L-cols;
                            # strictly-lower (T2) elsewhere (see reference mask)
                            didx = 0 if (jt <= 3 or (jt <= 7 and k == 1)) else 1
                            nc.vector.tensor_tensor(
                                pt[:jl, 0:P],
                                pt[:jl, 0:P],
                                maskD[:jl, didx, :],
                                mybir.AluOpType.mult,
                            )
                        elif jt == 12 and k < 2:
                            nc.vector.tensor_tensor(
                                pt[:jl, :ln],
                                pt[:jl, :ln],
                                maskTx[:jl, a - 512 : a - 512 + ln],
                                mybir.AluOpType.mult,
                            )
                        pts.append(pt)

                    # att@v: same stationary v tile for all chunks of this jt
                    for k, (a, ln) in enumerate(ivs):
                        parts = [(a, ln, 0)]
                        if a < 1536 < a + ln:  # merged torso+text interval
                            parts = [(a, 1536 - a, 0), (1536, a + ln - 1536, 1536 - a)]
                        for pa, pl, poff in parts:
                            ich = _ich_of(pa)
                            off = pa - (0, 512, 1024, 1536)[ich]
                            nc.tensor.matmul(
                                ps_y[ich][:, off : off + pl],
                                v_ones[:jl, jt, h, :],
                                pts[k][:jl, poff : poff + pl],
                                start=not started[ich],
                                stop=(jt == _ATTV_LAST[ich]),
                                skip_group_check=True,
                            )
                            started[ich] = True

                for ich, (i0, ilen) in enumerate(I_CHUNKS):
                    den = phB.tile([1, 512], F16, name="den", tag="den", bufs=4)
                    nc.vector.tensor_copy(den[0:1, :ilen], ps_y[ich][D : D + 1, :ilen])
                    ps_bc = psBC.tile([D, 512], F32, name="ps_bc", tag="ps_bc")
                    nc.tensor.matmul(
                        ps_bc[:, :ilen],
                        ones64[0:1, :],
                        den[0:1, :ilen],
                        start=True,
                        stop=True,
                    )
                    rc = phB.tile([D, 512], F32, name="rc", tag="rc", bufs=4)
                    nc.vector.reciprocal_approx_fast(out=rc[:, :ilen], in_=ps_bc[:, :ilen])
                    nc.vector.tensor_tensor(
                        yT[pof : pof + D, ct, i0 : i0 + ilen],
                        ps_y[ich][0:D, :ilen],
                        rc[:, :ilen],
                        mybir.AluOpType.mult,
                    )

        # ---------- Phase C: output projection (partial) ----------
        with (
            tc.tile_pool(name="phC", bufs=3) as phC,
            tc.tile_pool(name="psC", bufs=2, space="PSUM") as psC,
        ):
            for it in range(NJT):
                il = _jl(it)
                o_sb = phC.tile([P, C], F32, name="o_sb", tag="o_sb")
                for nch in range(2):
                    ps_o = psC.tile([P, 384], F32, name="ps_o", tag="ps_o")
                    for kt in range(CT):
                        nc.tensor.matmul(
                            ps_o[:il, :],
                            yT[:, kt, it * P : it * P + il],
                            wp_sb[:, kt, nch * 384 : (nch + 1) * 384],
                            start=(kt == 0),
                            stop=(kt == CT - 1),
                        )
                    nc.any.tensor_copy(o_sb[:il, nch * 384 : (nch + 1) * 384], ps_o[:il, :])
                nc.sync.dma_start(out_d[it * P : it * P + il, :], o_sb[:il, :])

    nc.compile()
    return nc


def _build_mask_np(seg_starts, seg_ends):
    """True = masked. Mirrors reference._build_mask in numpy."""
    ML = 3 * T
    tril = np.tril(np.ones((T, T), dtype=bool))
    sl = np.tril(np.ones((T, T), dtype=bool), -1)
    m = np.zeros((L, L), dtype=bool)
    m[:ML, :ML] = True
    m[0:T, 0:T] = ~tril
    m[T : 2 * T, 0:T] = ~tril
    m[T : 2 * T, T : 2 * T] = ~sl
    m[T : 2 * T, 2 * T : 3 * T] = ~sl
    m[2 * T : 3 * T, 0:T] = ~tril
    m[2 * T : 3 * T, T : 2 * T] = ~tril
    m[2 * T : 3 * T, 2 * T : 3 * T] = ~sl
    m[:ML, ML:] = True
    frames = np.arange(T)[None, :, None]
    allowed = (frames >= seg_starts[:, None, :]) & (frames < seg_ends[:, None, :])
    mask = np.broadcast_to(m[None], (B, L, L)).copy()
    for row0, col_blocks in ((T, (0, 2, 3)), (2 * T, (1, 2, 3))):
        for j in col_blocks:
            c0 = ML + j * N
            mask[:, row0 : row0 + T, c0 : c0 + N] &= ~allowed
    return mask


def get_nc():
    global _NC
    if _NC is None:
        _NC = _build_program()
    return _NC


def make_in_maps(x, Wq, bq, Wk, bk, Wv, bv, Wp, bp, seg_starts, seg_ends):
    mask = _build_mask_np(np.asarray(seg_starts), np.asarray(seg_ends))
    r = np.arange(P)
    maskD = np.empty((P, 2, P), dtype=np.float16)
    maskD[:, 0, :] = (r[:, None] <= r[None, :]).astype(np.float16)  # tril.T
    maskD[:, 1, :] = (r[:, None] < r[None, :]).astype(np.float16)  # strict
    in_maps = []
    for core in range(8):
        b, g = core // 2, core % 2
        gs = slice(g * G, (g + 1) * G)
        allowT = ~mask[b].T  # [j, i]
        maskTx = np.ascontiguousarray(
            allowT[1536:1568, 512:1536].astype(np.float16)
        )
        in_maps.append(
            {
                "xT": np.ascontiguousarray(x[b].T).astype(np.float16),
                "wqT": np.ascontiguousarray(Wq[gs, :].T).astype(np.float16),
                "wkT": np.ascontiguousarray(Wk[gs, :].T).astype(np.float16),
                "wvT": np.ascontiguousarray(Wv[gs, :].T).astype(np.float16),
                "wpT": np.ascontiguousarray(Wp[:, gs].T).astype(np.float16),
                "bqP": np.ascontiguousarray(bq[gs].reshape(CT, P).T),
                "bkP": np.ascontiguousarray(bk[gs].reshape(CT, P).T),
                "bvB": np.broadcast_to(bv[gs], (P, G)).copy(),
                "maskD": maskD,
                "maskTxt": maskTx,
            }
        )
    return in_maps


def kernel(x, Wq, bq, Wk, bk, Wv, bv, Wp, bp, seg_starts, seg_ends, T_motion=None,
           N=None, _trace=False, **_unused):
    x = np.asarray(x, np.float32)
    args = [np.asarray(a, np.float32) for a in (Wq, bq, Wk, bk, Wv, bv, Wp, bp)]
    Wq, bq, Wk, bk, Wv, bv, Wp, bp = args
    nc = get_nc()
    in_maps = make_in_maps(x, Wq, bq, Wk, bk, Wv, bv, Wp, bp, seg_starts, seg_ends)
    res = run_bass_kernel_spmd(nc, in_maps, core_ids=list(range(8)), trace=_trace)
    parts = [r["out_part"] for r in res.results]
    y = np.empty((B, L, C), np.float32)
    for b in range(B):
        y[b] = parts[2 * b] + parts[2 * b + 1] + bp
    if _trace:
        kernel.last_results = res
    return y
